# revision 10
# baseline (speedup 1.0000x reference)
"""Trainium2 Bass kernel for nn_AttentionUnit (B=4, S=2048, D=1024, H=16).

Sharding: 8 cores = 4 batches x 2 head-groups (Megatron column/row split).
Host->device traffic is minimized: each core receives only its half of the
batch's transposed q/k/v (d-rows hg*512..), plus a 1/4 chunk of its weight
halves; full per-core operands are reconstructed on-device with AllGather
(pairs {2b,2b+1} for activations, quads {0,2,4,6}/{1,3,5,7} for weights).
The two head-group partial outputs are pair-reduced on device with a bf16
ReduceScatter, so each core ships only [4, 256, 1024] bf16 back.

Per core (batch b, 8-head half hg):
  Q^T,K^T = (Wq/Wk half)^T-proj of inputs   [dh=512 on partitions, seq free]
  V       = natural [seq, dh=512] (+ ones column per head for softmax denom)
  S^T     = K @ Q^T / 8 (causal blocks skipped, padding via exp bias)
  P^T     = exp(S^T)  (unnormalized, bf16)
  O^T     = V_aug^T @ P^T  -> row 64 is the softmax denominator
  attn^T  = O^T[0:64] * recip(denom)  (reciprocals batched 8 heads/block)
  partial = attn @ Wo_half -> bf16 -> pair ReduceScatter(add) -> host

All matmuls bf16 with fp32 PSUM accumulation; softmax entirely fp32.
"""

import sys

sys.path.insert(0, "/opt/trn_rl_repo")

import numpy as np
import ml_dtypes

S = 2048
D = 1024
P = 128
DH = 64          # head dim
HPC = 8          # heads per core
DHH = 512        # dh per core (8 heads * 64)
QB = 512         # q block
NQB = S // QB    # 4
DMC = D // P     # 8 dmodel chunks
NT = S // P      # 16 k tiles
NEG_CAUSAL = -1.0e12   # added pre-scale (scale=0.125 applied inside exp)
NEG_PAD = -1.0e9       # added post-scale (exp bias)

PAIRS = [[0, 1], [2, 3], [4, 5], [6, 7]]
QUADS = [[0, 2, 4, 6], [1, 3, 5, 7]]

_CACHE = {}


def _build_program(kcap=NT, psum33=True, oldrecip=False, nors=False):
    import concourse.bass as bass
    import concourse.tile as tile
    from concourse import bacc, mybir

    f32 = mybir.dt.float32
    bf16 = mybir.dt.bfloat16
    ADD = mybir.AluOpType.add
    MUL = mybir.AluOpType.mult
    BYP = mybir.AluOpType.bypass
    EXP = mybir.ActivationFunctionType.Exp

    nc = bacc.Bacc("TRN2", target_bir_lowering=False, debug=False)

    # --- external I/O (minimized tunnel bytes) ---
    xin_d = nc.dram_tensor("xin", [NQB, 3, DHH, QB], bf16, kind="ExternalInput")
    wp_d = nc.dram_tensor("wp", [4, 256, DHH], bf16, kind="ExternalInput")
    padb_d = nc.dram_tensor("padb", [P, NT], f32, kind="ExternalInput")
    mask_d = nc.dram_tensor("mask", [P, P], f32, kind="ExternalInput")
    bq_d = nc.dram_tensor("bq", [P, 4], f32, kind="ExternalInput")
    bk_d = nc.dram_tensor("bk", [P, 4], f32, kind="ExternalInput")
    out_d = nc.dram_tensor("outp", [NQB, QB // 2, D], bf16, kind="ExternalOutput")

    with tile.TileContext(nc) as tc:
        with (
            tc.tile_pool(name="dram", bufs=1, space="DRAM") as dramp,
            tc.tile_pool(name="const", bufs=1) as constp,
            tc.tile_pool(name="inp", bufs=2) as inp,
            tc.tile_pool(name="probs", bufs=4) as probsp,
            tc.tile_pool(name="small", bufs=2) as smallp,
            tc.tile_pool(name="osb", bufs=2) as osbp,
            tc.tile_pool(name="psA", bufs=2, space="PSUM") as psA,
            tc.tile_pool(name="psB", bufs=2, space="PSUM") as psB,
            tc.tile_pool(name="psB3", bufs=3 if psum33 else 2, space="PSUM") as psB3,
            tc.tile_pool(name="psC", bufs=1 if psum33 else 2, space="PSUM") as psC,
        ):
            # ---- internal DRAM for collectives ----
            xb = [dramp.tile([3, DHH, QB], bf16, tag=f"xb{j}", name=f"xb{j}") for j in range(NQB)]
            xg = [dramp.tile([2, 3, DHH, QB], bf16, tag=f"xg{j}", name=f"xg{j}") for j in range(NQB)]
            wb = dramp.tile([4, 256, DHH], bf16, tag="wb")
            wg = dramp.tile([4, 4, 256, DHH], bf16, tag="wg")
            ub = [dramp.tile([QB, D], bf16, tag=f"ub{j}", name=f"ub{j}") for j in range(NQB)]
            ur = [dramp.tile([QB // 2, D], bf16, tag=f"ur{j}", name=f"ur{j}") for j in range(NQB)]

            # bounce + gather: first x chunk and weights first (critical path)
            nc.gpsimd.dma_start(xb[0][:], xin_d[0])
            nc.gpsimd.collective_compute(
                "AllGather", BYP, replica_groups=PAIRS,
                ins=[xb[0].opt()], outs=[xg[0].opt()],
            )
            nc.gpsimd.dma_start(wb[:], wp_d[:])
            nc.gpsimd.collective_compute(
                "AllGather", BYP, replica_groups=QUADS,
                ins=[wb.opt()], outs=[wg.opt()],
            )
            for j in range(1, NQB):
                nc.gpsimd.dma_start(xb[j][:], xin_d[j])
                nc.gpsimd.collective_compute(
                    "AllGather", BYP, replica_groups=PAIRS,
                    ins=[xb[j].opt()], outs=[xg[j].opt()],
                )

            # ---- persistent SBUF tensors ----
            wq_sb = constp.tile([P, DMC, DHH], bf16, tag="wq")
            wk_sb = constp.tile([P, DMC, DHH], bf16, tag="wk")
            wv_sb = constp.tile([P, DMC, DHH], bf16, tag="wv")
            wo_sb = constp.tile([P, 4, D], bf16, tag="wo")
            QT = constp.tile([P, 4, S], bf16, tag="QT")
            KT = constp.tile([P, 4, S], bf16, tag="KT")
            V = constp.tile([P, NT, HPC, DH + 1], bf16, tag="V")
            AT = constp.tile([P, 4, S], bf16, tag="AT")
            padb = constp.tile([P, NT], f32, tag="padb")
            mask = constp.tile([P, P], f32, tag="mask")
            bq_sb = constp.tile([P, 4], f32, tag="bq")
            bk_sb = constp.tile([P, 4], f32, tag="bk")

            # weight SBUF loads from gathered chunks
            for wsb, slab in ((wq_sb, 0), (wk_sb, 1), (wv_sb, 2)):
                for m in range(4):
                    nc.sync.dma_start(
                        wsb[:, 2 * m : 2 * m + 2, :],
                        wg[m, slab].rearrange("(two p) h -> p two h", p=P),
                    )
            for m in range(4):
                nc.sync.dma_start(
                    wo_sb[:, m, :],
                    wg[m, 3].rearrange("(p two) c -> p (two c)", p=P),
                )
            nc.sync.dma_start(padb[:], padb_d[:])
            nc.sync.dma_start(mask[:], mask_d[:])
            nc.sync.dma_start(bq_sb[:], bq_d[:])
            nc.sync.dma_start(bk_sb[:], bk_d[:])
            # ones column for the softmax denominator
            nc.vector.memset(V[:, :, :, DH : DH + 1], 1.0)

            for qb in range(NQB):
                q0 = qb * QB
                kn = min(QB, max(0, kcap * P - q0))
                # ======== phase A: projections for this block ========
                xgv = xg[qb].rearrange("g t (c4 p) s -> t p g c4 s", p=P)
                qin = inp.tile([P, DMC, QB], bf16, tag="qin")
                kin = inp.tile([P, DMC, QB], bf16, tag="kin")
                vin = inp.tile([P, DMC, QB], bf16, tag="vin")
                for xt, t in ((qin, 0), (kin, 1), (vin, 2)):
                    for g in range(2):
                        nc.sync.dma_start(xt[:, 4 * g : 4 * g + 4, :], xgv[t, :, g])

                for c in range(4):
                    ps = psA.tile([P, QB], f32, tag="proj")
                    for mc in range(DMC):
                        nc.tensor.matmul(
                            ps[:],
                            wq_sb[:, mc, c * P : (c + 1) * P],
                            qin[:, mc, :],
                            start=(mc == 0),
                            stop=(mc == DMC - 1),
                        )
                    nc.vector.tensor_scalar(
                        QT[:, c, q0 : q0 + QB], ps[:], bq_sb[:, c : c + 1], None, ADD
                    )
                    if kn > 0:
                        ps = psA.tile([P, QB], f32, tag="proj")
                        for mc in range(DMC):
                            nc.tensor.matmul(
                                ps[:, :kn],
                                wk_sb[:, mc, c * P : (c + 1) * P],
                                kin[:, mc, :kn],
                                start=(mc == 0),
                                stop=(mc == DMC - 1),
                            )
                        nc.vector.tensor_scalar(
                            KT[:, c, q0 : q0 + kn], ps[:, :kn], bk_sb[:, c : c + 1], None, ADD
                        )
                for j in range(4):
                    kt_i = 4 * qb + j
                    if kt_i >= kcap:
                        continue
                    ps = psA.tile([P, QB], f32, tag="proj")
                    for mc in range(DMC):
                        nc.tensor.matmul(
                            ps[:],
                            vin[:, mc, j * P : (j + 1) * P],
                            wv_sb[:, mc, :],
                            start=(mc == 0),
                            stop=(mc == DMC - 1),
                        )
                    nc.vector.tensor_copy(
                        V[:, kt_i, :, 0:DH],
                        ps[:].rearrange("p (h d) -> p h d", d=DH),
                    )

                # ======== phase B: attention for this q block ========
                kt_max = min(4 * qb + 4, kcap)
                den = smallp.tile([HPC, QB], f32, tag="den")
                recd = smallp.tile([HPC, QB], f32, tag="recd")
                _ = (den, recd)
                for h in range(HPC):
                    po = (h % 2) * DH
                    ch = h // 2
                    pv = psB.tile([DH + 1, QB], f32, tag="pv")
                    for kt in range(kt_max):
                        delta = max(0, P * kt - q0)
                        sc = psB3.tile([P, QB], f32, tag="sc")
                        nc.tensor.matmul(
                            sc[:, delta:],
                            KT[po : po + DH, ch, kt * P : (kt + 1) * P],
                            QT[po : po + DH, ch, q0 + delta : q0 + QB],
                            start=True,
                            stop=True,
                        )
                        if kt >= 4 * qb:
                            nc.vector.tensor_tensor(
                                sc[:, delta : delta + P],
                                sc[:, delta : delta + P],
                                mask[:],
                                ADD,
                            )
                        pr = probsp.tile([P, QB], bf16, tag="probs")
                        nc.scalar.activation(
                            pr[:, delta:],
                            sc[:, delta:],
                            EXP,
                            bias=padb[:, kt : kt + 1],
                            scale=0.125,
                        )
                        nc.tensor.matmul(
                            pv[:, delta:],
                            V[:, kt, h, :],
                            pr[:, delta:],
                            start=(kt == 0),
                            stop=(kt == kt_max - 1),
                        )
                    if oldrecip:
                        rec = smallp.tile([DH + 1, QB], f32, tag="rec")
                        nc.vector.reciprocal(rec[DH : DH + 1, :], pv[DH : DH + 1, :])
                        rec0 = smallp.tile([1, QB], f32, tag="rec0")
                        nc.sync.dma_start(rec0[0:1, :], rec[DH : DH + 1, :])
                        rep1 = smallp.tile([P, QB], f32, tag="rep1")
                        nc.gpsimd.partition_broadcast(rep1[0:DH, :], rec0[0:1, :])
                        if po == 0:
                            nc.vector.tensor_tensor(
                                AT[0:DH, ch, q0 : q0 + QB], pv[0:DH, :], rep1[0:DH, :], MUL
                            )
                        else:
                            tmpo = smallp.tile([DH, QB], bf16, tag="tmpo")
                            nc.vector.tensor_tensor(
                                tmpo[:], pv[0:DH, :], rep1[0:DH, :], MUL
                            )
                            nc.sync.dma_start(
                                AT[po : po + DH, ch, q0 : q0 + QB], tmpo[:]
                            )
                        continue
                    # move unnormalized attn + denominator out of PSUM
                    # (DMA cannot read PSUM: stage via vector copies)
                    dtmp = smallp.tile([DH + 1, QB], f32, tag="dtmp")
                    nc.vector.tensor_copy(dtmp[DH : DH + 1, :], pv[DH : DH + 1, :])
                    nc.sync.dma_start(den[h : h + 1, :], dtmp[DH : DH + 1, :])
                    if po == 0:
                        nc.vector.tensor_copy(
                            AT[0:DH, ch, q0 : q0 + QB], pv[0:DH, :]
                        )
                    else:
                        tmp = smallp.tile([DH, QB], bf16, tag="tmp")
                        nc.vector.tensor_copy(tmp[:], pv[0:DH, :])
                        nc.sync.dma_start(
                            AT[po : po + DH, ch, q0 : q0 + QB], tmp[:]
                        )
                # batched reciprocal for all 8 heads of this block
                if not oldrecip:
                    nc.vector.reciprocal(recd[:], den[:])
                    for ch in range(4):
                        r0 = smallp.tile([1, QB], f32, tag="r0")
                        rep = smallp.tile([P, QB], f32, tag="rep")
                        nc.sync.dma_start(r0[0:1, :], recd[2 * ch : 2 * ch + 1, :])
                        nc.gpsimd.partition_broadcast(rep[0:DH, :], r0[0:1, :])
                        r1 = smallp.tile([1, QB], f32, tag="r1")
                        nc.sync.dma_start(r1[0:1, :], recd[2 * ch + 1 : 2 * ch + 2, :])
                        rep2 = smallp.tile([DH, QB], f32, tag="rep2")
                        nc.gpsimd.partition_broadcast(rep2[0:DH, :], r1[0:1, :])
                        nc.sync.dma_start(rep[DH:P, :], rep2[0:DH, :])
                        nc.vector.tensor_tensor(
                            AT[:, ch, q0 : q0 + QB],
                            AT[:, ch, q0 : q0 + QB],
                            rep[:],
                            MUL,
                        )

                # ======== phase C: output projection for this q block ========
                for j in range(4):
                    qt_i = 4 * qb + j
                    osb = osbp.tile([P, D], bf16, tag="osb")
                    for half in range(2):
                        fin = psC.tile([P, QB], f32, tag="fin")
                        for c in range(4):
                            lhsT = AT[:, c, qt_i * P : (qt_i + 1) * P]
                            nc.tensor.matmul(
                                fin[:],
                                lhsT,
                                wo_sb[:, c, half * 512 : half * 512 + 512],
                                start=(c == 0),
                                stop=(c == 3),
                            )
                        nc.vector.tensor_copy(
                            osb[:, half * 512 : half * 512 + 512], fin[:]
                        )
                    nc.sync.dma_start(ub[qb][j * P : (j + 1) * P, :], osb[:])
                # pair-reduce this block's partials on device
                nc.gpsimd.collective_compute(
                    "ReduceScatter", ADD, replica_groups=PAIRS,
                    ins=[ub[qb].opt()], outs=[ur[qb].opt()],
                )
                nc.gpsimd.dma_start(out_d[qb], ur[qb][:])

    nc.compile()
    return nc


def _get_program(kcap=NT, **flags):
    key = (kcap, tuple(sorted(flags.items())))
    if key not in _CACHE:
        _CACHE[key] = _build_program(kcap=kcap, **flags)
    return _CACHE[key]


def _make_in_maps(q_input, k_input, v_input, key_padding_mask, Wq, Wk, Wv, Wo, bq, bk):
    bf = ml_dtypes.bfloat16
    mask128 = np.where(
        np.arange(P)[None, :] < np.arange(P)[:, None], NEG_CAUSAL, 0.0
    ).astype(np.float32)
    xs = (q_input, k_input, v_input)
    in_maps = []
    for core in range(8):
        b = core // 2
        hg = core % 2
        sl = slice(hg * DHH, (hg + 1) * DHH)
        padv = np.where(key_padding_mask[b], NEG_PAD, 0.0).astype(np.float32)
        xin = np.empty((NQB, 3, DHH, QB), dtype=bf)
        for t in range(3):
            slT = xs[t][b][:, sl].T  # [DHH, S] view
            for j in range(NQB):
                xin[j, t] = slT[:, j * QB : (j + 1) * QB]
        wp = np.empty((4, 256, DHH), dtype=bf)
        wp[0] = Wq[b * 256 : (b + 1) * 256, sl]
        wp[1] = Wk[b * 256 : (b + 1) * 256, sl]
        wp[2] = Wv[b * 256 : (b + 1) * 256, sl]
        wp[3] = np.ascontiguousarray(
            Wo[hg * DHH + b * P : hg * DHH + (b + 1) * P, :]
        ).reshape(256, DHH)
        in_maps.append(
            {
                "xin": xin,
                "wp": wp,
                "padb": np.ascontiguousarray(padv.reshape(NT, P).T),
                "mask": mask128,
                "bq": np.ascontiguousarray(bq[sl].reshape(4, P).T.astype(np.float32)),
                "bk": np.ascontiguousarray(bk[sl].reshape(4, P).T.astype(np.float32)),
            }
        )
    return in_maps


def run_spmd(in_maps, kcap=NT, flags=None, **kwargs):
    from concourse import bass_utils

    nc = _get_program(kcap=kcap, **(flags or {}))
    return bass_utils.run_bass_kernel_spmd(
        nc, in_maps, core_ids=list(range(8)), **kwargs
    )


def kernel(q_input, k_input, v_input, key_padding_mask,
           Wq, bq, Wk, bk, Wv, bv, Wo, bo, **_unused):
    q_input = np.asarray(q_input, dtype=np.float32)
    k_input = np.asarray(k_input, dtype=np.float32)
    v_input = np.asarray(v_input, dtype=np.float32)
    key_padding_mask = np.asarray(key_padding_mask)
    in_maps = _make_in_maps(
        q_input, k_input, v_input, key_padding_mask,
        np.asarray(Wq, np.float32), np.asarray(Wk, np.float32),
        np.asarray(Wv, np.float32), np.asarray(Wo, np.float32),
        np.asarray(bq, np.float32), np.asarray(bk, np.float32),
    )
    valid = S - key_padding_mask.astype(np.int64).sum(axis=1)
    kcap = int(min(NT, max(1, -(-int(valid.max()) // P))))
    res = run_spmd(in_maps, kcap=kcap).results
    bo = np.asarray(bo, np.float32)
    bv = np.asarray(bv, np.float32)
    # bv support: normalized attention plus bv equals attn output with biased V
    # (rows of softmax sum to 1) -> fold bv through Wo into the output bias.
    extra = bv @ np.asarray(Wo, np.float32) if np.any(bv) else 0.0
    out = np.empty((4, S, D), np.float32)
    for b in range(4):
        ov = out[b].reshape(NQB, 2, QB // 2, D)
        ov[:, 0] = res[2 * b]["outp"].astype(np.float32)
        ov[:, 1] = res[2 * b + 1]["outp"].astype(np.float32)
    out += bo + extra
    return out


# revision 15
# speedup vs baseline: 1.1782x; 1.1782x over previous
"""Trainium2 Bass kernel for nn_AttentionUnit (B=4, S=2048, D=1024, H=16).

Sharding: 8 cores = 4 batches x 2 head-groups (Megatron column/row split).

I/O strategy (minimizes host->device bytes while keeping the NEFF critical
path collective-free at the start):
  - weights and seq-chunk 0 of q/k/v ship in full per core (local compute
    for q-block 0 starts immediately; the first collective's one-time
    ~130us init cost is hidden under that compute),
  - seq-chunks 1-3 ship as halves (d-rows hg*512..) and are reconstructed
    with pair AllGathers {2b,2b+1} that complete in the background,
  - the two head-group partials are pair-reduced on device with a bf16
    ReduceScatter, so each core ships only [4, 256, 1024] bf16 back.

Per core (batch b, 8-head half hg):
  Q^T,K^T = (Wq/Wk half)^T-proj of inputs   [dh=512 on partitions, seq free]
  V       = natural [seq, dh=512] (+ ones column per head for softmax denom)
  S^T     = K @ Q^T / 8 (causal blocks skipped, padding via exp bias)
  P^T     = exp(S^T)  (unnormalized, bf16)
  O^T     = V_aug^T @ P^T  -> row 64 is the softmax denominator
  attn^T  = O^T[0:64] * recip(denom)  (reciprocals batched 8 heads/block,
            broadcast across partitions via a tiny K=2 select-matmul)
  partial = attn @ Wo_half -> bf16 -> pair ReduceScatter(add) -> host

Phases are emitted as A(0) B(0) A(1) norm(0) C(0) B(1) ... so the tensor
engine fills the normalize-chain latency with the next block's projections.
All matmuls bf16 with fp32 PSUM accumulation; softmax entirely fp32.
"""

import sys

sys.path.insert(0, "/opt/trn_rl_repo")

import numpy as np
import ml_dtypes

S = 2048
D = 1024
P = 128
DH = 64          # head dim
HPC = 8          # heads per core
DHH = 512        # dh per core (8 heads * 64)
QB = 512         # q block
NQB = S // QB    # 4
DMC = D // P     # 8 dmodel chunks
NT = S // P      # 16 k tiles
NEG_CAUSAL = -1.0e12   # added pre-scale (scale=0.125 applied inside exp)
NEG_PAD = -1.0e9       # added post-scale (exp bias)

PAIRS = [[0, 1], [2, 3], [4, 5], [6, 7]]

_CACHE = {}


def _build_program(kcap=NT):
    import concourse.bass as bass
    import concourse.tile as tile
    from concourse import bacc, mybir

    f32 = mybir.dt.float32
    bf16 = mybir.dt.bfloat16
    ADD = mybir.AluOpType.add
    MUL = mybir.AluOpType.mult
    BYP = mybir.AluOpType.bypass
    EXP = mybir.ActivationFunctionType.Exp

    nc = bacc.Bacc("TRN2", target_bir_lowering=False, debug=False)

    # --- external I/O ---
    x0_d = nc.dram_tensor("x0", [3, D, QB], bf16, kind="ExternalInput")
    xin_d = nc.dram_tensor("xin", [3, 3, DHH, QB], bf16, kind="ExternalInput")
    wq_d = nc.dram_tensor("wq", [D, DHH], bf16, kind="ExternalInput")
    wk_d = nc.dram_tensor("wk", [D, DHH], bf16, kind="ExternalInput")
    wv_d = nc.dram_tensor("wv", [D, DHH], bf16, kind="ExternalInput")
    wo_d = nc.dram_tensor("wo", [DHH, D], bf16, kind="ExternalInput")
    padb_d = nc.dram_tensor("padb", [P, NT], f32, kind="ExternalInput")
    mask_d = nc.dram_tensor("mask", [P, P], f32, kind="ExternalInput")
    bq_d = nc.dram_tensor("bq", [P, 4], f32, kind="ExternalInput")
    sel_d = nc.dram_tensor("sel", [2, P], bf16, kind="ExternalInput")
    bk_d = nc.dram_tensor("bk", [P, 4], f32, kind="ExternalInput")
    out_d = nc.dram_tensor("outp", [NQB, QB // 2, D], bf16, kind="ExternalOutput")

    with tile.TileContext(nc) as tc:
        with (
            tc.tile_pool(name="dram", bufs=1, space="DRAM") as dramp,
            tc.tile_pool(name="const", bufs=1) as constp,
            tc.tile_pool(name="inp", bufs=2) as inp,
            tc.tile_pool(name="probs", bufs=4) as probsp,
            tc.tile_pool(name="small", bufs=2) as smallp,
            tc.tile_pool(name="osb", bufs=2) as osbp,
            tc.tile_pool(name="psA", bufs=2, space="PSUM") as psA,
            tc.tile_pool(name="psB", bufs=2, space="PSUM") as psB,
            tc.tile_pool(name="psB3", bufs=3, space="PSUM") as psB3,
            tc.tile_pool(name="psC", bufs=1, space="PSUM") as psC,
        ):
            # ---- internal DRAM for collectives (chunks 1-3 + output) ----
            xb = [dramp.tile([3, DHH, QB], bf16, tag=f"xb{j}", name=f"xb{j}")
                  for j in range(1, NQB)]
            xg = [dramp.tile([2, 3, DHH, QB], bf16, tag=f"xg{j}", name=f"xg{j}")
                  for j in range(1, NQB)]
            ub = [dramp.tile([QB, D], bf16, tag=f"ub{j}", name=f"ub{j}")
                  for j in range(NQB)]
            ur = [dramp.tile([QB // 2, D], bf16, tag=f"ur{j}", name=f"ur{j}")
                  for j in range(NQB)]

            # background gathers for chunks 1-3 (first cc pays comm init,
            # hidden under local q-block-0 compute)
            for j in range(NQB - 1):
                nc.gpsimd.dma_start(xb[j][:], xin_d[j])
                nc.gpsimd.collective_compute(
                    "AllGather", BYP, replica_groups=PAIRS,
                    ins=[xb[j].opt()], outs=[xg[j].opt()],
                )

            # ---- persistent SBUF tensors ----
            wq_sb = constp.tile([P, DMC, DHH], bf16, tag="wq")
            wk_sb = constp.tile([P, DMC, DHH], bf16, tag="wk")
            wv_sb = constp.tile([P, DMC, DHH], bf16, tag="wv")
            wo_sb = constp.tile([P, 4, D], bf16, tag="wo")
            QT = constp.tile([P, 4, S], bf16, tag="QT")
            KT = constp.tile([P, 4, S], bf16, tag="KT")
            V = constp.tile([P, NT, HPC, DH + 1], bf16, tag="V")
            AT = constp.tile([P, 4, S], bf16, tag="AT")
            padb = constp.tile([P, NT], f32, tag="padb")
            mask = constp.tile([P, P], f32, tag="mask")
            bq_sb = constp.tile([P, 4], f32, tag="bq")
            bk_sb = constp.tile([P, 4], f32, tag="bk")
            sel = constp.tile([2, P], bf16, tag="sel")

            nc.sync.dma_start(wq_sb[:], wq_d.rearrange("(c p) m -> p c m", p=P))
            nc.sync.dma_start(wk_sb[:], wk_d.rearrange("(c p) m -> p c m", p=P))
            nc.sync.dma_start(wv_sb[:], wv_d.rearrange("(c p) m -> p c m", p=P))
            nc.sync.dma_start(wo_sb[:], wo_d.rearrange("(c p) m -> p c m", p=P))
            nc.sync.dma_start(padb[:], padb_d[:])
            nc.sync.dma_start(mask[:], mask_d[:])
            nc.sync.dma_start(bq_sb[:], bq_d[:])
            nc.sync.dma_start(bk_sb[:], bk_d[:])
            # ones column for the softmax denominator
            nc.vector.memset(V[:, :, :, DH : DH + 1], 1.0)
            # head-parity selector for the recip partition-broadcast matmul
            nc.sync.dma_start(sel[:], sel_d[:])

            def phase_a(qb):
                q0 = qb * QB
                kn = min(QB, max(0, kcap * P - q0))
                qin = inp.tile([P, DMC, QB], bf16, tag="qin", name="qin")
                kin = inp.tile([P, DMC, QB], bf16, tag="kin", name="kin")
                vin = inp.tile([P, DMC, QB], bf16, tag="vin", name="vin")
                if qb == 0:
                    x0v = x0_d.rearrange("t (c p) s -> t p c s", p=P)
                    nc.sync.dma_start(qin[:], x0v[0])
                    nc.sync.dma_start(kin[:], x0v[1])
                    nc.sync.dma_start(vin[:], x0v[2])
                else:
                    xgv = xg[qb - 1].rearrange("g t (c4 p) s -> t p g c4 s", p=P)
                    for xt, t in ((qin, 0), (kin, 1), (vin, 2)):
                        for g in range(2):
                            nc.sync.dma_start(xt[:, 4 * g : 4 * g + 4, :], xgv[t, :, g])

                for c in range(4):
                    ps = psA.tile([P, QB], f32, tag="proj", name="ps")
                    for mc in range(DMC):
                        nc.tensor.matmul(
                            ps[:],
                            wq_sb[:, mc, c * P : (c + 1) * P],
                            qin[:, mc, :],
                            start=(mc == 0),
                            stop=(mc == DMC - 1),
                        )
                    nc.vector.tensor_scalar(
                        QT[:, c, q0 : q0 + QB], ps[:], bq_sb[:, c : c + 1], None, ADD
                    )
                    if kn > 0:
                        ps = psA.tile([P, QB], f32, tag="proj", name="ps")
                        for mc in range(DMC):
                            nc.tensor.matmul(
                                ps[:, :kn],
                                wk_sb[:, mc, c * P : (c + 1) * P],
                                kin[:, mc, :kn],
                                start=(mc == 0),
                                stop=(mc == DMC - 1),
                            )
                        nc.vector.tensor_scalar(
                            KT[:, c, q0 : q0 + kn], ps[:, :kn], bk_sb[:, c : c + 1], None, ADD
                        )
                for j in range(4):
                    kt_i = 4 * qb + j
                    if kt_i >= kcap:
                        continue
                    ps = psA.tile([P, QB], f32, tag="proj", name="ps")
                    for mc in range(DMC):
                        nc.tensor.matmul(
                            ps[:],
                            vin[:, mc, j * P : (j + 1) * P],
                            wv_sb[:, mc, :],
                            start=(mc == 0),
                            stop=(mc == DMC - 1),
                        )
                    nc.vector.tensor_copy(
                        V[:, kt_i, :, 0:DH],
                        ps[:].rearrange("p (h d) -> p h d", d=DH),
                    )

            def phase_b(qb, den):
                q0 = qb * QB
                kt_max = min(4 * qb + 4, kcap)
                for h in range(HPC):
                    po = (h % 2) * DH
                    ch = h // 2
                    pv = psB.tile([DH + 1, QB], f32, tag="pv", name="pv")
                    for kt in range(kt_max):
                        delta = max(0, P * kt - q0)
                        sc = psB3.tile([P, QB], f32, tag="sc", name="sc")
                        nc.tensor.matmul(
                            sc[:, delta:],
                            KT[po : po + DH, ch, kt * P : (kt + 1) * P],
                            QT[po : po + DH, ch, q0 + delta : q0 + QB],
                            start=True,
                            stop=True,
                        )
                        if kt >= 4 * qb:
                            nc.vector.tensor_tensor(
                                sc[:, delta : delta + P],
                                sc[:, delta : delta + P],
                                mask[:],
                                ADD,
                            )
                        pr = probsp.tile([P, QB], bf16, tag="probs", name="pr")
                        nc.scalar.activation(
                            pr[:, delta:],
                            sc[:, delta:],
                            EXP,
                            bias=padb[:, kt : kt + 1],
                            scale=0.125,
                        )
                        nc.tensor.matmul(
                            pv[:, delta:],
                            V[:, kt, h, :],
                            pr[:, delta:],
                            start=(kt == 0),
                            stop=(kt == kt_max - 1),
                        )
                    # move unnormalized attn + denominator out of PSUM
                    # (DMA cannot read PSUM: stage via vector copies)
                    dtmp = smallp.tile([DH + 1, QB], f32, tag="dtmp", name="dtmp")
                    nc.vector.tensor_copy(dtmp[DH : DH + 1, :], pv[DH : DH + 1, :])
                    nc.sync.dma_start(den[h : h + 1, :], dtmp[DH : DH + 1, :])
                    if po == 0:
                        nc.vector.tensor_copy(AT[0:DH, ch, q0 : q0 + QB], pv[0:DH, :])
                    else:
                        tmp = smallp.tile([DH, QB], bf16, tag="tmp", name="tmp")
                        nc.vector.tensor_copy(tmp[:], pv[0:DH, :])
                        nc.sync.dma_start(AT[po : po + DH, ch, q0 : q0 + QB], tmp[:])

            def normalize(qb, den):
                q0 = qb * QB
                recd = smallp.tile([HPC, QB], bf16, tag="recd", name="recd")
                recd2 = smallp.tile([2, 4, QB], bf16, tag="recd2", name="recd2")
                with nc.allow_low_precision(reason="bf16 recip feeds bf16 attn normalize"):
                    nc.vector.reciprocal(recd[:], den[:])
                for h in range(HPC):
                    nc.sync.dma_start(
                        recd2[h % 2 : h % 2 + 1, h // 2, :], recd[h : h + 1, :]
                    )
                for ch in range(4):
                    rep = psB3.tile([P, QB], f32, tag="sc", name="rep")
                    nc.tensor.matmul(
                        rep[:], sel[:, :], recd2[:, ch, :], start=True, stop=True
                    )
                    nc.vector.tensor_tensor(
                        AT[:, ch, q0 : q0 + QB],
                        AT[:, ch, q0 : q0 + QB],
                        rep[:],
                        MUL,
                    )

            def phase_c(qb):
                for j in range(4):
                    qt_i = 4 * qb + j
                    osb = osbp.tile([P, D], bf16, tag="osb", name="osb")
                    for half in range(2):
                        fin = psC.tile([P, QB], f32, tag="fin", name="fin")
                        for c in range(4):
                            lhsT = AT[:, c, qt_i * P : (qt_i + 1) * P]
                            nc.tensor.matmul(
                                fin[:],
                                lhsT,
                                wo_sb[:, c, half * 512 : half * 512 + 512],
                                start=(c == 0),
                                stop=(c == 3),
                            )
                        nc.vector.tensor_copy(
                            osb[:, half * 512 : half * 512 + 512], fin[:]
                        )
                    nc.sync.dma_start(ub[qb][j * P : (j + 1) * P, :], osb[:])
                nc.gpsimd.collective_compute(
                    "ReduceScatter", ADD, replica_groups=PAIRS,
                    ins=[ub[qb].opt()], outs=[ur[qb].opt()],
                )
                nc.gpsimd.dma_start(out_d[qb], ur[qb][:])

            phase_a(0)
            for qb in range(NQB):
                den = smallp.tile([HPC, QB], f32, tag=f"den{qb % 2}", name="den")
                phase_b(qb, den)
                if qb + 1 < NQB:
                    phase_a(qb + 1)
                normalize(qb, den)
                phase_c(qb)

    nc.compile()
    return nc


def _get_program(kcap=NT):
    if kcap not in _CACHE:
        _CACHE[kcap] = _build_program(kcap=kcap)
    return _CACHE[kcap]


def _sel_const():
    bf = ml_dtypes.bfloat16
    sel = np.zeros((2, P), dtype=bf)
    sel[0, 0:DH] = 1.0
    sel[1, DH:P] = 1.0
    return sel


def _make_in_maps(q_input, k_input, v_input, key_padding_mask, Wq, Wk, Wv, Wo, bq, bk):
    bf = ml_dtypes.bfloat16
    mask128 = np.where(
        np.arange(P)[None, :] < np.arange(P)[:, None], NEG_CAUSAL, 0.0
    ).astype(np.float32)
    xs = (q_input, k_input, v_input)
    in_maps = []
    for core in range(8):
        b = core // 2
        hg = core % 2
        sl = slice(hg * DHH, (hg + 1) * DHH)
        padv = np.where(key_padding_mask[b], NEG_PAD, 0.0).astype(np.float32)
        x0 = np.empty((3, D, QB), dtype=bf)
        xin = np.empty((3, 3, DHH, QB), dtype=bf)
        for t in range(3):
            xT = xs[t][b].T  # [D, S] view
            x0[t] = xT[:, 0:QB]
            half = xT[sl]
            for j in range(1, NQB):
                xin[j - 1, t] = half[:, j * QB : (j + 1) * QB]
        in_maps.append(
            {
                "x0": x0,
                "xin": xin,
                "wq": Wq[:, sl].astype(bf),
                "wk": Wk[:, sl].astype(bf),
                "wv": Wv[:, sl].astype(bf),
                "wo": np.ascontiguousarray(Wo[sl, :]).astype(bf),
                "padb": np.ascontiguousarray(padv.reshape(NT, P).T),
                "mask": mask128,
                "bq": np.ascontiguousarray(bq[sl].reshape(4, P).T.astype(np.float32)),
                "sel": _sel_const(),
                "bk": np.ascontiguousarray(bk[sl].reshape(4, P).T.astype(np.float32)),
            }
        )
    return in_maps


def run_spmd(in_maps, kcap=NT, **kwargs):
    from concourse import bass_utils

    nc = _get_program(kcap=kcap)
    return bass_utils.run_bass_kernel_spmd(
        nc, in_maps, core_ids=list(range(8)), **kwargs
    )


def kernel(q_input, k_input, v_input, key_padding_mask,
           Wq, bq, Wk, bk, Wv, bv, Wo, bo, **_unused):
    q_input = np.asarray(q_input, dtype=np.float32)
    k_input = np.asarray(k_input, dtype=np.float32)
    v_input = np.asarray(v_input, dtype=np.float32)
    key_padding_mask = np.asarray(key_padding_mask)
    in_maps = _make_in_maps(
        q_input, k_input, v_input, key_padding_mask,
        np.asarray(Wq, np.float32), np.asarray(Wk, np.float32),
        np.asarray(Wv, np.float32), np.asarray(Wo, np.float32),
        np.asarray(bq, np.float32), np.asarray(bk, np.float32),
    )
    valid = S - key_padding_mask.astype(np.int64).sum(axis=1)
    kcap = int(min(NT, max(1, -(-int(valid.max()) // P))))
    res = run_spmd(in_maps, kcap=kcap).results
    bo = np.asarray(bo, np.float32)
    bv = np.asarray(bv, np.float32)
    # bv support: normalized attention plus bv equals attn output with biased V
    # (rows of softmax sum to 1) -> fold bv through Wo into the output bias.
    extra = bv @ np.asarray(Wo, np.float32) if np.any(bv) else 0.0
    out = np.empty((4, S, D), np.float32)
    for b in range(4):
        ov = out[b].reshape(NQB, 2, QB // 2, D)
        ov[:, 0] = res[2 * b]["outp"].astype(np.float32)
        ov[:, 1] = res[2 * b + 1]["outp"].astype(np.float32)
    out += bo + extra
    return out


# revision 17
# speedup vs baseline: 1.5276x; 1.2966x over previous
"""Trainium2 Bass kernel for nn_AttentionUnit (B=4, S=2048, D=1024, H=16).

Sharding: 8 cores = 4 batches x 2 head-groups (Megatron column/row split).

I/O strategy (minimizes host->device bytes while keeping the NEFF critical
path collective-free at the start):
  - weights and seq-chunk 0 of q/k/v ship in full per core (local compute
    for q-block 0 starts immediately; the first collective's one-time
    ~130us init cost is hidden under that compute),
  - seq-chunks 1-3 ship as halves (d-rows hg*512..) and are reconstructed
    with pair AllGathers {2b,2b+1} that complete in the background,
  - the two head-group partials are pair-reduced on device with a bf16
    ReduceScatter, so each core ships only [4, 256, 1024] bf16 back.

Per core (batch b, 8-head half hg):
  Q^T,K^T = (Wq/Wk half)^T-proj of inputs   [dh=512 on partitions, seq free]
  V       = natural [seq, dh=512] (+ ones column per head for softmax denom)
  S^T     = K @ Q^T / 8 (causal blocks skipped, padding via exp bias)
  P^T     = exp(S^T)  (unnormalized, bf16)
  O^T     = V_aug^T @ P^T  -> row 64 is the softmax denominator
  attn^T  = O^T[0:64] * recip(denom)  (reciprocals batched 8 heads/block,
            broadcast across partitions via a tiny K=2 select-matmul)
  partial = attn @ Wo_half -> bf16 -> pair ReduceScatter(add) -> host

Phases are emitted as A(0) B(0) A(1) norm(0) C(0) B(1) ... so the tensor
engine fills the normalize-chain latency with the next block's projections.
All matmuls bf16 with fp32 PSUM accumulation; softmax entirely fp32.
"""

import sys

sys.path.insert(0, "/opt/trn_rl_repo")

import numpy as np
import ml_dtypes

S = 2048
D = 1024
P = 128
DH = 64          # head dim
HPC = 8          # heads per core
DHH = 512        # dh per core (8 heads * 64)
QB = 512         # q block
NQB = S // QB    # 4
DMC = D // P     # 8 dmodel chunks
NT = S // P      # 16 k tiles
NEG_CAUSAL = -1.0e12   # added pre-scale (scale=0.125 applied inside exp)
NEG_PAD = -1.0e9       # added post-scale (exp bias)

PAIRS = [[0, 1], [2, 3], [4, 5], [6, 7]]

_CACHE = {}


def _build_program(kcap=NT):
    import concourse.bass as bass
    import concourse.tile as tile
    from concourse import bacc, mybir

    f32 = mybir.dt.float32
    bf16 = mybir.dt.bfloat16
    ADD = mybir.AluOpType.add
    MUL = mybir.AluOpType.mult
    BYP = mybir.AluOpType.bypass
    EXP = mybir.ActivationFunctionType.Exp

    nc = bacc.Bacc("TRN2", target_bir_lowering=False, debug=False)

    # --- external I/O ---
    x0_d = nc.dram_tensor("x0", [2, 3, D, QB], bf16, kind="ExternalInput")
    xin_d = nc.dram_tensor("xin", [2, 3, DHH, QB], bf16, kind="ExternalInput")
    wq_d = nc.dram_tensor("wq", [D, DHH], bf16, kind="ExternalInput")
    wk_d = nc.dram_tensor("wk", [D, DHH], bf16, kind="ExternalInput")
    wv_d = nc.dram_tensor("wv", [D, DHH], bf16, kind="ExternalInput")
    wo_d = nc.dram_tensor("wo", [DHH, D], bf16, kind="ExternalInput")
    padb_d = nc.dram_tensor("padb", [P, NT], f32, kind="ExternalInput")
    mask_d = nc.dram_tensor("mask", [P, P], f32, kind="ExternalInput")
    bq_d = nc.dram_tensor("bq", [P, 4], f32, kind="ExternalInput")
    sel_d = nc.dram_tensor("sel", [2, P], bf16, kind="ExternalInput")
    bk_d = nc.dram_tensor("bk", [P, 4], f32, kind="ExternalInput")
    out_d = nc.dram_tensor("outp", [S, D], bf16, kind="ExternalOutput")

    with tile.TileContext(nc) as tc:
        with (
            tc.tile_pool(name="dram", bufs=1, space="DRAM") as dramp,
            tc.tile_pool(name="const", bufs=1) as constp,
            tc.tile_pool(name="inp", bufs=2) as inp,
            tc.tile_pool(name="probs", bufs=4) as probsp,
            tc.tile_pool(name="small", bufs=2) as smallp,
            tc.tile_pool(name="osb", bufs=2) as osbp,
            tc.tile_pool(name="psA", bufs=2, space="PSUM") as psA,
            tc.tile_pool(name="psB", bufs=2, space="PSUM") as psB,
            tc.tile_pool(name="psB3", bufs=3, space="PSUM") as psB3,
            tc.tile_pool(name="psC", bufs=1, space="PSUM") as psC,
        ):
            # ---- internal DRAM for collectives (chunks 1-3 + output) ----
            xb = [dramp.tile([3, DHH, QB], bf16, tag=f"xb{j}", name=f"xb{j}")
                  for j in range(2)]
            xg = [dramp.tile([2, 3, DHH, QB], bf16, tag=f"xg{j}", name=f"xg{j}")
                  for j in range(2)]

            # background gathers for chunks 2-3 (first cc pays comm init,
            # hidden under local q-block-0/1 compute)
            for j in range(2):
                nc.gpsimd.dma_start(xb[j][:], xin_d[j])
                nc.gpsimd.collective_compute(
                    "AllGather", BYP, replica_groups=PAIRS,
                    ins=[xb[j].opt()], outs=[xg[j].opt()],
                )

            # ---- persistent SBUF tensors ----
            wq_sb = constp.tile([P, DMC, DHH], bf16, tag="wq")
            wk_sb = constp.tile([P, DMC, DHH], bf16, tag="wk")
            wv_sb = constp.tile([P, DMC, DHH], bf16, tag="wv")
            wo_sb = constp.tile([P, 4, D], bf16, tag="wo")
            QT = constp.tile([P, 4, S], bf16, tag="QT")
            KT = constp.tile([P, 4, S], bf16, tag="KT")
            V = constp.tile([P, NT, HPC, DH + 1], bf16, tag="V")
            AT = constp.tile([P, 4, S], bf16, tag="AT")
            padb = constp.tile([P, NT], f32, tag="padb")
            mask = constp.tile([P, P], f32, tag="mask")
            bq_sb = constp.tile([P, 4], f32, tag="bq")
            bk_sb = constp.tile([P, 4], f32, tag="bk")
            sel = constp.tile([2, P], bf16, tag="sel")

            nc.sync.dma_start(wq_sb[:], wq_d.rearrange("(c p) m -> p c m", p=P))
            # ones column for the softmax denominator
            nc.vector.memset(V[:, :, :, DH : DH + 1], 1.0)

            def phase_a(qb):
                q0 = qb * QB
                kn = min(QB, max(0, kcap * P - q0))
                qin = inp.tile([P, DMC, QB], bf16, tag="qin", name="qin")
                kin = inp.tile([P, DMC, QB], bf16, tag="kin", name="kin")
                vin = inp.tile([P, DMC, QB], bf16, tag="vin", name="vin")
                if qb < 2:
                    x0v = x0_d[qb].rearrange("t (c p) s -> t p c s", p=P)
                    nc.sync.dma_start(qin[:], x0v[0])
                    if qb == 0:
                        nc.sync.dma_start(wk_sb[:], wk_d.rearrange("(c p) m -> p c m", p=P))
                    nc.sync.dma_start(kin[:], x0v[1])
                    if qb == 0:
                        nc.sync.dma_start(wv_sb[:], wv_d.rearrange("(c p) m -> p c m", p=P))
                    nc.sync.dma_start(vin[:], x0v[2])
                    if qb == 0:
                        nc.sync.dma_start(padb[:], padb_d[:])
                        nc.sync.dma_start(mask[:], mask_d[:])
                        nc.sync.dma_start(bq_sb[:], bq_d[:])
                        nc.sync.dma_start(bk_sb[:], bk_d[:])
                        nc.sync.dma_start(sel[:], sel_d[:])
                        nc.sync.dma_start(
                            wo_sb[:], wo_d.rearrange("(c p) m -> p c m", p=P)
                        )
                else:
                    xgv = xg[qb - 2].rearrange("g t (c4 p) s -> t p g c4 s", p=P)
                    for xt, t in ((qin, 0), (kin, 1), (vin, 2)):
                        for g in range(2):
                            nc.sync.dma_start(xt[:, 4 * g : 4 * g + 4, :], xgv[t, :, g])

                for c in range(4):
                    ps = psA.tile([P, QB], f32, tag="proj", name="ps")
                    for mc in range(DMC):
                        nc.tensor.matmul(
                            ps[:],
                            wq_sb[:, mc, c * P : (c + 1) * P],
                            qin[:, mc, :],
                            start=(mc == 0),
                            stop=(mc == DMC - 1),
                        )
                    nc.vector.tensor_scalar(
                        QT[:, c, q0 : q0 + QB], ps[:], bq_sb[:, c : c + 1], None, ADD
                    )
                    if kn > 0:
                        ps = psA.tile([P, QB], f32, tag="proj", name="ps")
                        for mc in range(DMC):
                            nc.tensor.matmul(
                                ps[:, :kn],
                                wk_sb[:, mc, c * P : (c + 1) * P],
                                kin[:, mc, :kn],
                                start=(mc == 0),
                                stop=(mc == DMC - 1),
                            )
                        nc.vector.tensor_scalar(
                            KT[:, c, q0 : q0 + kn], ps[:, :kn], bk_sb[:, c : c + 1], None, ADD
                        )
                for j in range(4):
                    kt_i = 4 * qb + j
                    if kt_i >= kcap:
                        continue
                    ps = psA.tile([P, QB], f32, tag="proj", name="ps")
                    for mc in range(DMC):
                        nc.tensor.matmul(
                            ps[:],
                            vin[:, mc, j * P : (j + 1) * P],
                            wv_sb[:, mc, :],
                            start=(mc == 0),
                            stop=(mc == DMC - 1),
                        )
                    nc.vector.tensor_copy(
                        V[:, kt_i, :, 0:DH],
                        ps[:].rearrange("p (h d) -> p h d", d=DH),
                    )

            def phase_b(qb, den):
                q0 = qb * QB
                kt_max = min(4 * qb + 4, kcap)
                for h in range(HPC):
                    po = (h % 2) * DH
                    ch = h // 2
                    pv = psB.tile([DH + 1, QB], f32, tag="pv", name="pv")
                    for kt in range(kt_max):
                        delta = max(0, P * kt - q0)
                        sc = psB3.tile([P, QB], f32, tag="sc", name="sc")
                        nc.tensor.matmul(
                            sc[:, delta:],
                            KT[po : po + DH, ch, kt * P : (kt + 1) * P],
                            QT[po : po + DH, ch, q0 + delta : q0 + QB],
                            start=True,
                            stop=True,
                        )
                        if kt >= 4 * qb:
                            nc.vector.tensor_tensor(
                                sc[:, delta : delta + P],
                                sc[:, delta : delta + P],
                                mask[:],
                                ADD,
                            )
                        pr = probsp.tile([P, QB], bf16, tag="probs", name="pr")
                        nc.scalar.activation(
                            pr[:, delta:],
                            sc[:, delta:],
                            EXP,
                            bias=padb[:, kt : kt + 1],
                            scale=0.125,
                        )
                        nc.tensor.matmul(
                            pv[:, delta:],
                            V[:, kt, h, :],
                            pr[:, delta:],
                            start=(kt == 0),
                            stop=(kt == kt_max - 1),
                        )
                    # move unnormalized attn + denominator out of PSUM
                    # (DMA cannot read PSUM: stage via vector copies)
                    dtmp = smallp.tile([DH + 1, QB], f32, tag="dtmp", name="dtmp")
                    nc.vector.tensor_copy(dtmp[DH : DH + 1, :], pv[DH : DH + 1, :])
                    nc.sync.dma_start(den[h : h + 1, :], dtmp[DH : DH + 1, :])
                    if po == 0:
                        nc.vector.tensor_copy(AT[0:DH, ch, q0 : q0 + QB], pv[0:DH, :])
                    else:
                        tmp = smallp.tile([DH, QB], bf16, tag="tmp", name="tmp")
                        nc.vector.tensor_copy(tmp[:], pv[0:DH, :])
                        nc.sync.dma_start(AT[po : po + DH, ch, q0 : q0 + QB], tmp[:])

            def normalize(qb, den):
                q0 = qb * QB
                recd = smallp.tile([HPC, QB], bf16, tag="recd", name="recd")
                recd2 = smallp.tile([2, 4, QB], bf16, tag="recd2", name="recd2")
                with nc.allow_low_precision(reason="bf16 recip feeds bf16 attn normalize"):
                    nc.vector.reciprocal(recd[:], den[:])
                for h in range(HPC):
                    nc.sync.dma_start(
                        recd2[h % 2 : h % 2 + 1, h // 2, :], recd[h : h + 1, :]
                    )
                for ch in range(4):
                    rep = psB3.tile([P, QB], f32, tag="sc", name="rep")
                    nc.tensor.matmul(
                        rep[:], sel[:, :], recd2[:, ch, :], start=True, stop=True
                    )
                    nc.vector.tensor_tensor(
                        AT[:, ch, q0 : q0 + QB],
                        AT[:, ch, q0 : q0 + QB],
                        rep[:],
                        MUL,
                    )

            def phase_c(qb):
                for j in range(4):
                    qt_i = 4 * qb + j
                    osb = osbp.tile([P, D], bf16, tag="osb", name="osb")
                    for half in range(2):
                        fin = psC.tile([P, QB], f32, tag="fin", name="fin")
                        for c in range(4):
                            lhsT = AT[:, c, qt_i * P : (qt_i + 1) * P]
                            nc.tensor.matmul(
                                fin[:],
                                lhsT,
                                wo_sb[:, c, half * 512 : half * 512 + 512],
                                start=(c == 0),
                                stop=(c == 3),
                            )
                        nc.vector.tensor_copy(
                            osb[:, half * 512 : half * 512 + 512], fin[:]
                        )
                    nc.sync.dma_start(out_d[qt_i * P : (qt_i + 1) * P, :], osb[:])

            phase_a(0)
            for qb in range(NQB):
                den = smallp.tile([HPC, QB], f32, tag=f"den{qb % 2}", name="den")
                phase_b(qb, den)
                if qb + 1 < NQB:
                    phase_a(qb + 1)
                normalize(qb, den)
                phase_c(qb)

    nc.compile()
    return nc


def _get_program(kcap=NT):
    if kcap not in _CACHE:
        _CACHE[kcap] = _build_program(kcap=kcap)
    return _CACHE[kcap]


def _sel_const():
    bf = ml_dtypes.bfloat16
    sel = np.zeros((2, P), dtype=bf)
    sel[0, 0:DH] = 1.0
    sel[1, DH:P] = 1.0
    return sel


def _make_in_maps(q_input, k_input, v_input, key_padding_mask, Wq, Wk, Wv, Wo, bq, bk):
    bf = ml_dtypes.bfloat16
    mask128 = np.where(
        np.arange(P)[None, :] < np.arange(P)[:, None], NEG_CAUSAL, 0.0
    ).astype(np.float32)
    xs = (q_input, k_input, v_input)
    in_maps = []
    for core in range(8):
        b = core // 2
        hg = core % 2
        sl = slice(hg * DHH, (hg + 1) * DHH)
        padv = np.where(key_padding_mask[b], NEG_PAD, 0.0).astype(np.float32)
        x0 = np.empty((2, 3, D, QB), dtype=bf)
        xin = np.empty((2, 3, DHH, QB), dtype=bf)
        for t in range(3):
            xT = xs[t][b].T  # [D, S] view
            x0[0, t] = xT[:, 0:QB]
            x0[1, t] = xT[:, QB : 2 * QB]
            half = xT[sl]
            for j in range(2, NQB):
                xin[j - 2, t] = half[:, j * QB : (j + 1) * QB]
        in_maps.append(
            {
                "x0": x0,
                "xin": xin,
                "wq": Wq[:, sl].astype(bf),
                "wk": Wk[:, sl].astype(bf),
                "wv": Wv[:, sl].astype(bf),
                "wo": np.ascontiguousarray(Wo[sl, :]).astype(bf),
                "padb": np.ascontiguousarray(padv.reshape(NT, P).T),
                "mask": mask128,
                "bq": np.ascontiguousarray(bq[sl].reshape(4, P).T.astype(np.float32)),
                "sel": _sel_const(),
                "bk": np.ascontiguousarray(bk[sl].reshape(4, P).T.astype(np.float32)),
            }
        )
    return in_maps


def run_spmd(in_maps, kcap=NT, **kwargs):
    from concourse import bass_utils

    nc = _get_program(kcap=kcap)
    return bass_utils.run_bass_kernel_spmd(
        nc, in_maps, core_ids=list(range(8)), **kwargs
    )


def kernel(q_input, k_input, v_input, key_padding_mask,
           Wq, bq, Wk, bk, Wv, bv, Wo, bo, **_unused):
    q_input = np.asarray(q_input, dtype=np.float32)
    k_input = np.asarray(k_input, dtype=np.float32)
    v_input = np.asarray(v_input, dtype=np.float32)
    key_padding_mask = np.asarray(key_padding_mask)
    in_maps = _make_in_maps(
        q_input, k_input, v_input, key_padding_mask,
        np.asarray(Wq, np.float32), np.asarray(Wk, np.float32),
        np.asarray(Wv, np.float32), np.asarray(Wo, np.float32),
        np.asarray(bq, np.float32), np.asarray(bk, np.float32),
    )
    valid = S - key_padding_mask.astype(np.int64).sum(axis=1)
    kcap = int(min(NT, max(1, -(-int(valid.max()) // P))))
    res = run_spmd(in_maps, kcap=kcap).results
    bo = np.asarray(bo, np.float32)
    bv = np.asarray(bv, np.float32)
    # bv support: normalized attention plus bv equals attn output with biased V
    # (rows of softmax sum to 1) -> fold bv through Wo into the output bias.
    extra = bv @ np.asarray(Wo, np.float32) if np.any(bv) else 0.0
    out = np.empty((4, S, D), np.float32)
    for b in range(4):
        out[b] = res[2 * b]["outp"].astype(np.float32)
        out[b] += res[2 * b + 1]["outp"].astype(np.float32)
    out += bo + extra
    return out
